# revision 23
# baseline (speedup 1.0000x reference)
"""Self-contained Trainium2 kernel for nn_AttentionHelper (B=16, C=128, L=2048).

reference:
    energy = einsum("bcq,bck->bqk", Q, K) / sqrt(C)
    att    = softmax(energy + log(mask + 1e-6), axis=-1) * mask
    out    = einsum("bck,bqk->bcq", V, att)
    return out, att.transpose(0, 2, 1)

Strategy (data-parallel over batch, 2 batches per core on 8 cores):
  - Q/K/V are cast to bf16 during the DMA load (SWDGE cast); PE streams bf16
    at twice the fp32 rate and fp32 PSUM accumulation keeps the contraction
    accurate (~4e-3 overall, gate is 2e-2).
  - E^T = K^T Q computed directly in [k, q] layout (lhsT = K chunk, rhs = Q).
  - One ACT pass: P = exp(E^T/sqrt(C) + 2*log(m+eps)) with per-partition bias
    (k is the partition axis in this layout, so the key mask is a bias AP).
    The doubled log folds the post-softmax multiplicative mask into the same
    pass: (m+eps)^2 = m*(m+eps) up to 1e-6 relative. P is stored bf16.
  - denom[q] = sum_k P via PE matmul with all-ones lhsT (M=1, 4 col-tiled
    concurrent accumulators). Masked keys contribute ~1e-7 relative, so the
    exact 1/(m+eps) weighting of the reference denominator is unnecessary.
  - 1/denom is broadcast across partitions with a PE "select" matmul
    (lhsT = one-hot row matrix), which also discards the junk partitions.
  - A^T = P * (1/denom)[q] broadcast: elementwise multiply (bf16) split
    between DVE and GPSIMD, in place, then each [128, 2048] chunk is DMA'd
    out (bf16 DRAM, cast back to f32 on the host).
  - out = V @ A^T accumulated over k chunks (lhsT = V^T via PE transposes),
    in two q-halves so PSUM stays within 8 banks.
  - Emission is software-pipelined: phase 2 of batch b is interleaved with
    phase 1 of batch b+1 so the PE/ACT/DVE queues alternate between batches.
"""

import math
import sys
from contextlib import ExitStack

if "/opt/trn_rl_repo" not in sys.path:
    sys.path.insert(0, "/opt/trn_rl_repo")

import numpy as np

import concourse.bass as bass  # noqa: F401  (engine types referenced via nc)
import concourse.mybir as mybir
import concourse.tile as tile
from concourse import bacc
from concourse.bass_utils import run_bass_kernel_spmd
from concourse.masks import make_identity

B, C, L = 16, 128, 2048
N_CORES = 8
BPC = B // N_CORES  # batches per core
NK = L // 128  # 16 key chunks of 128
SCALE = 1.0 / math.sqrt(C)
EPS = 1e-6

F32 = mybir.dt.float32
BF16 = mybir.dt.bfloat16
AF = mybir.ActivationFunctionType

_CACHE: dict = {}
LAST_RESULTS = None


class _BatchState:
    pass


def _build():
    nc = bacc.Bacc("TRN2", target_bir_lowering=False, num_devices=N_CORES)

    q_d = nc.declare_dram_parameter("q", [BPC, C, L], F32, isOutput=False)
    k_d = nc.declare_dram_parameter("k", [BPC, C, L], F32, isOutput=False)
    v_d = nc.declare_dram_parameter("v", [BPC, C, L], F32, isOutput=False)
    m_d = nc.declare_dram_parameter("mask", [BPC, 1, L], F32, isOutput=False)
    out_d = nc.declare_dram_parameter("out", [BPC, C, L], F32, isOutput=True)
    at_d = nc.declare_dram_parameter("attn_t", [BPC, L, L], BF16, isOutput=True)

    with tile.TileContext(nc) as tc, ExitStack() as ctx:
        sing = ctx.enter_context(tc.tile_pool(name="singles", bufs=1))
        qp = ctx.enter_context(tc.tile_pool(name="qp", bufs=2))
        kp = ctx.enter_context(tc.tile_pool(name="kp", bufs=2))
        vp = ctx.enter_context(tc.tile_pool(name="vp", bufs=2))
        vtp = ctx.enter_context(tc.tile_pool(name="vtp", bufs=2))
        rbp = ctx.enter_context(tc.tile_pool(name="rbp", bufs=2))
        outp = ctx.enter_context(tc.tile_pool(name="outp", bufs=1))
        maskp = ctx.enter_context(tc.tile_pool(name="maskp", bufs=2))
        stagep = ctx.enter_context(tc.tile_pool(name="stagep", bufs=1))
        p2tp = ctx.enter_context(tc.tile_pool(name="p2t", bufs=2 * NK))
        ps_et = ctx.enter_context(tc.tile_pool(name="ps_et", bufs=2, space="PSUM"))
        ps_dn = ctx.enter_context(tc.tile_pool(name="ps_dn", bufs=1, space="PSUM"))
        ps_vt = ctx.enter_context(tc.tile_pool(name="ps_vt", bufs=1, space="PSUM"))
        ps_out = ctx.enter_context(tc.tile_pool(name="ps_out", bufs=1, space="PSUM"))

        ident = sing.tile([128, 128], BF16)
        make_identity(nc, ident)
        ones_col = sing.tile([128, 1], BF16, tag="ones_col")
        nc.vector.memset(ones_col[:], 1.0)
        # sel[n][p, m] = 1 if p == 32n else 0 — "pick row 32n" via matmul
        sels = []
        for n in range(4):
            sel = sing.tile([128, 128], BF16, tag=f"sel{n}")
            nc.vector.memset(sel[:], 0.0)
            nc.vector.memset(sel[32 * n : 32 * n + 1, :], 1.0)
            sels.append(sel)

        st = [_BatchState() for _ in range(BPC)]

        def emit_inputs(b):
            s = st[b]
            s.m_cols = maskp.tile([128, NK], F32, tag="m_cols")
            nc.sync.dma_start(
                s.m_cols[:], m_d.ap()[b].rearrange("o (j p) -> (o p) j", p=128)
            )
            # fp32 loads into staging on three separate DGE queues (they
            # would serialize on one), then DVE cast to bf16
            engines = {"k": nc.sync, "q": nc.sync, "v": nc.gpsimd}
            for name, dram in (("k", k_d), ("q", q_d), ("v", v_d)):
                stg = stagep.tile([C, L], F32, tag=f"stg_{name}")
                engines[name].dma_start(stg[:], dram.ap()[b])
                bf = {"q": qp, "k": kp, "v": vp}[name].tile([C, L], BF16)
                nc.vector.tensor_copy(bf[:], stg[:])
                setattr(s, name, bf)

        def emit_maskprep(b):
            s = st[b]
            mpe = maskp.tile([128, NK], F32, tag="mpe")
            nc.vector.tensor_scalar_add(mpe[:], s.m_cols[:], EPS)
            s.logm2 = maskp.tile([128, NK], F32, tag="logm2")
            nc.scalar.activation(s.logm2[:], mpe[:], AF.Ln)
            nc.scalar.mul(s.logm2[:], s.logm2[:], 2.0)
            s.dn_ps = ps_dn.tile([128, 512], F32)
            # unused partitions must stay finite: 0 * inf = NaN would leak
            # through the select matmul in emit_boundary
            nc.vector.memset(s.dn_ps[:], 1.0)
            s.chunks = []

        def emit_vt(b):
            # V^T (vt[p, 128*kb + c] = V[c, 128*kb + p])
            s = st[b]
            s.vt = vtp.tile([128, L], BF16)
            for g in range(4):
                pvt = ps_vt.tile([128, 512], BF16)
                for j in range(4):
                    kb = 4 * g + j
                    nc.tensor.transpose(
                        pvt[:, j * 128 : (j + 1) * 128],
                        s.v[:, kb * 128 : (kb + 1) * 128],
                        ident[:],
                    )
                nc.vector.tensor_copy(s.vt[:, g * 512 : (g + 1) * 512], pvt[:])

        def emit_et_exp(b, kb):
            s = st[b]
            p2t = p2tp.tile([128, L], BF16)
            s.chunks.append(p2t)
            for h in range(2):
                et = ps_et.tile([128, 1024], F32, tag="et")
                for n in range(2):
                    nc.tensor.matmul(
                        et[:, n * 512 : (n + 1) * 512],
                        lhsT=s.k[:, kb * 128 : (kb + 1) * 128],
                        rhs=s.q[:, h * 1024 + n * 512 : h * 1024 + (n + 1) * 512],
                        start=True,
                        stop=True,
                    )
                nc.scalar.activation(
                    p2t[:, h * 1024 : (h + 1) * 1024],
                    et[:],
                    AF.Exp,
                    bias=s.logm2[:, kb : kb + 1],
                    scale=SCALE,
                )

        def emit_dn(b, kb):
            s = st[b]
            for n in range(4):
                nc.tensor.matmul(
                    s.dn_ps[32 * n : 32 * n + 1, :],
                    lhsT=ones_col[:],
                    rhs=s.chunks[kb][:, n * 512 : (n + 1) * 512],
                    start=(kb == 0),
                    stop=(kb == NK - 1),
                    tile_position=(0, 32 * n),
                )

        def emit_ph1_chunk(b, kb):
            # ET/exp of chunk kb, then the denominator of chunk kb-1: the dn
            # matmuls wait on exp, so keep ready ET work ahead of them in the
            # in-order PE queue.
            emit_et_exp(b, kb)
            if kb >= 1:
                emit_dn(b, kb - 1)
            if kb == NK - 1:
                emit_dn(b, kb)

        def emit_boundary(b):
            # r = 1/denom (slices live at partitions 0/32/64/96 of dn_ps; the
            # wide reciprocal computes junk on unused partitions, which the
            # select matmul then discards), cast bf16, broadcast via PE select.
            s = st[b]
            rec32 = maskp.tile([128, 512], F32, tag="rec32")
            nc.vector.reciprocal(rec32[:], s.dn_ps[:])
            rec = maskp.tile([128, 512], BF16, tag="rec")
            nc.scalar.copy(rec[:], rec32[:])
            s.rbc = rbp.tile([128, L], BF16)
            for h in range(2):
                rb_ps = ps_et.tile([128, 1024], F32, tag="et")
                for j in range(2):
                    n = 2 * h + j
                    nc.tensor.matmul(
                        rb_ps[:, j * 512 : (j + 1) * 512],
                        lhsT=sels[n][:],
                        rhs=rec[:],
                        start=True,
                        stop=True,
                    )
                nc.scalar.copy(s.rbc[:, h * 1024 : (h + 1) * 1024], rb_ps[:])

        def emit_rmul_dma(b, kb):
            s = st[b]
            p2t = s.chunks[kb]
            nc.vector.tensor_mul(p2t[:], p2t[:], s.rbc[:])
            nc.sync.dma_start(at_d.ap()[b, kb * 128 : (kb + 1) * 128, :], p2t[:])

        def emit_av_h0(b, kb):
            s = st[b]
            if kb == 0:
                s.out_ps = ps_out.tile([128, 1024], F32, tag="out")
            for n in range(2):
                nc.tensor.matmul(
                    s.out_ps[:, n * 512 : (n + 1) * 512],
                    lhsT=s.vt[:, kb * 128 : (kb + 1) * 128],
                    rhs=s.chunks[kb][:, n * 512 : (n + 1) * 512],
                    start=(kb == 0),
                    stop=(kb == NK - 1),
                )

        AV_LAG = 3

        def emit_ph2_chunk(b, kb):
            # r-multiply of chunk kb, then the AV matmuls of chunk kb-AV_LAG:
            # the AV matmuls wait on the DVE multiply; a deep stagger keeps
            # their waits satisfied by the time they reach the PE queue head.
            emit_rmul_dma(b, kb)
            if kb >= AV_LAG:
                emit_av_h0(b, kb - AV_LAG)
            if kb == NK - 1:
                for j in range(NK - AV_LAG, NK):
                    emit_av_h0(b, j)

        def emit_av_h1_and_out(b):
            s = st[b]
            osb = outp.tile([128, L], F32)
            nc.scalar.copy(osb[:, 0:1024], s.out_ps[:])
            out_ps1 = ps_out.tile([128, 1024], F32, tag="out")
            for kb in range(NK):
                for n in range(2):
                    nc.tensor.matmul(
                        out_ps1[:, n * 512 : (n + 1) * 512],
                        lhsT=s.vt[:, kb * 128 : (kb + 1) * 128],
                        rhs=s.chunks[kb][:, 1024 + n * 512 : 1024 + (n + 1) * 512],
                        start=(kb == 0),
                        stop=(kb == NK - 1),
                    )
            nc.scalar.copy(osb[:, 1024:2048], out_ps1[:])
            nc.sync.dma_start(out_d.ap()[b], osb[:])

        # software-pipelined emission
        emit_inputs(0)
        emit_maskprep(0)
        emit_inputs(1)
        for kb in range(NK):
            emit_ph1_chunk(0, kb)
            if kb == 2:
                emit_vt(0)
        emit_boundary(0)
        emit_maskprep(1)
        for kb in range(NK):
            emit_ph2_chunk(0, kb)
            emit_ph1_chunk(1, kb)
            if kb == 2:
                emit_vt(1)
        emit_av_h1_and_out(0)
        emit_boundary(1)
        for kb in range(NK):
            emit_ph2_chunk(1, kb)
        emit_av_h1_and_out(1)

    nc.compile()
    return nc


def kernel(proj_query, proj_key, proj_val, padding_mask):
    global LAST_RESULTS
    if "nc" not in _CACHE:
        _CACHE["nc"] = _build()
    nc = _CACHE["nc"]

    proj_query = np.ascontiguousarray(np.asarray(proj_query, dtype=np.float32))
    proj_key = np.ascontiguousarray(np.asarray(proj_key, dtype=np.float32))
    proj_val = np.ascontiguousarray(np.asarray(proj_val, dtype=np.float32))
    padding_mask = np.ascontiguousarray(np.asarray(padding_mask, dtype=np.float32))

    in_maps = []
    for i in range(N_CORES):
        s = slice(i * BPC, (i + 1) * BPC)
        in_maps.append(
            {
                "q": proj_query[s],
                "k": proj_key[s],
                "v": proj_val[s],
                "mask": padding_mask[s],
            }
        )

    res = run_bass_kernel_spmd(nc, in_maps, list(range(N_CORES)))
    LAST_RESULTS = res

    out = np.concatenate([res.results[i]["out"] for i in range(N_CORES)], axis=0)
    attn_t = np.concatenate(
        [np.asarray(res.results[i]["attn_t"], dtype=np.float32) for i in range(N_CORES)],
        axis=0,
    )
    return out, attn_t


# revision 24
# speedup vs baseline: 1.0145x; 1.0145x over previous
"""Self-contained Trainium2 kernel for nn_AttentionHelper (B=16, C=128, L=2048).

reference:
    energy = einsum("bcq,bck->bqk", Q, K) / sqrt(C)
    att    = softmax(energy + log(mask + 1e-6), axis=-1) * mask
    out    = einsum("bck,bqk->bcq", V, att)
    return out, att.transpose(0, 2, 1)

Strategy (data-parallel over batch, 2 batches per core on 8 cores):
  - Q/K/V are cast to bf16 during the DMA load (SWDGE cast); PE streams bf16
    at twice the fp32 rate and fp32 PSUM accumulation keeps the contraction
    accurate (~4e-3 overall, gate is 2e-2).
  - E^T = K^T Q computed directly in [k, q] layout (lhsT = K chunk, rhs = Q).
  - One ACT pass: P = exp(E^T/sqrt(C) + 2*log(m+eps)) with per-partition bias
    (k is the partition axis in this layout, so the key mask is a bias AP).
    The doubled log folds the post-softmax multiplicative mask into the same
    pass: (m+eps)^2 = m*(m+eps) up to 1e-6 relative. P is stored bf16.
  - denom[q] = sum_k P via PE matmul with all-ones lhsT (M=1, 4 col-tiled
    concurrent accumulators). Masked keys contribute ~1e-7 relative, so the
    exact 1/(m+eps) weighting of the reference denominator is unnecessary.
  - 1/denom is broadcast across partitions with a PE "select" matmul
    (lhsT = one-hot row matrix), which also discards the junk partitions.
  - A^T = P * (1/denom)[q] broadcast: elementwise multiply (bf16) split
    between DVE and GPSIMD, in place, then each [128, 2048] chunk is DMA'd
    out (bf16 DRAM, cast back to f32 on the host).
  - out = V @ A^T accumulated over k chunks (lhsT = V^T via PE transposes),
    in two q-halves so PSUM stays within 8 banks.
  - Emission is software-pipelined: phase 2 of batch b is interleaved with
    phase 1 of batch b+1 so the PE/ACT/DVE queues alternate between batches.
"""

import math
import sys
from contextlib import ExitStack

if "/opt/trn_rl_repo" not in sys.path:
    sys.path.insert(0, "/opt/trn_rl_repo")

import numpy as np

import concourse.bass as bass  # noqa: F401  (engine types referenced via nc)
import concourse.mybir as mybir
import concourse.tile as tile
from concourse import bacc
from concourse.bass_utils import run_bass_kernel_spmd
from concourse.masks import make_identity

B, C, L = 16, 128, 2048
N_CORES = 8
BPC = B // N_CORES  # batches per core
NK = L // 128  # 16 key chunks of 128
SCALE = 1.0 / math.sqrt(C)
EPS = 1e-6

F32 = mybir.dt.float32
BF16 = mybir.dt.bfloat16
AF = mybir.ActivationFunctionType

_CACHE: dict = {}
LAST_RESULTS = None


class _BatchState:
    pass


def _build():
    nc = bacc.Bacc("TRN2", target_bir_lowering=False, num_devices=N_CORES)

    q_d = nc.declare_dram_parameter("q", [BPC, C, L], F32, isOutput=False)
    k_d = nc.declare_dram_parameter("k", [BPC, C, L], F32, isOutput=False)
    v_d = nc.declare_dram_parameter("v", [BPC, C, L], F32, isOutput=False)
    m_d = nc.declare_dram_parameter("mask", [BPC, 1, L], F32, isOutput=False)
    out_d = nc.declare_dram_parameter("out", [BPC, C, L], F32, isOutput=True)
    at_d = nc.declare_dram_parameter("attn_t", [BPC, L, L], BF16, isOutput=True)

    with tile.TileContext(nc) as tc, ExitStack() as ctx:
        sing = ctx.enter_context(tc.tile_pool(name="singles", bufs=1))
        qp = ctx.enter_context(tc.tile_pool(name="qp", bufs=2))
        kp = ctx.enter_context(tc.tile_pool(name="kp", bufs=2))
        vp = ctx.enter_context(tc.tile_pool(name="vp", bufs=2))
        vtp = ctx.enter_context(tc.tile_pool(name="vtp", bufs=2))
        rbp = ctx.enter_context(tc.tile_pool(name="rbp", bufs=2))
        outp = ctx.enter_context(tc.tile_pool(name="outp", bufs=2))
        maskp = ctx.enter_context(tc.tile_pool(name="maskp", bufs=2))
        stagep = ctx.enter_context(tc.tile_pool(name="stagep", bufs=1))
        p2tp = ctx.enter_context(tc.tile_pool(name="p2t", bufs=2 * NK - 2))
        ps_et = ctx.enter_context(tc.tile_pool(name="ps_et", bufs=2, space="PSUM"))
        ps_dn = ctx.enter_context(tc.tile_pool(name="ps_dn", bufs=1, space="PSUM"))
        ps_vt = ctx.enter_context(tc.tile_pool(name="ps_vt", bufs=1, space="PSUM"))
        ps_out = ctx.enter_context(tc.tile_pool(name="ps_out", bufs=1, space="PSUM"))

        ident = sing.tile([128, 128], BF16)
        make_identity(nc, ident)
        ones_col = sing.tile([128, 1], BF16, tag="ones_col")
        nc.vector.memset(ones_col[:], 1.0)
        # sel[n][p, m] = 1 if p == 32n else 0 — "pick row 32n" via matmul
        sels = []
        for n in range(4):
            sel = sing.tile([128, 128], BF16, tag=f"sel{n}")
            nc.vector.memset(sel[:], 0.0)
            nc.vector.memset(sel[32 * n : 32 * n + 1, :], 1.0)
            sels.append(sel)

        st = [_BatchState() for _ in range(BPC)]

        def emit_inputs(b):
            s = st[b]
            s.m_cols = maskp.tile([128, NK], F32, tag="m_cols")
            nc.sync.dma_start(
                s.m_cols[:], m_d.ap()[b].rearrange("o (j p) -> (o p) j", p=128)
            )
            # fp32 loads into staging on three separate DGE queues (they
            # would serialize on one), then DVE cast to bf16
            engines = {"k": nc.sync, "q": nc.sync, "v": nc.gpsimd}
            for name, dram in (("k", k_d), ("q", q_d), ("v", v_d)):
                stg = stagep.tile([C, L], F32, tag=f"stg_{name}")
                engines[name].dma_start(stg[:], dram.ap()[b])
                bf = {"q": qp, "k": kp, "v": vp}[name].tile([C, L], BF16)
                nc.vector.tensor_copy(bf[:], stg[:])
                setattr(s, name, bf)

        def emit_maskprep(b):
            s = st[b]
            mpe = maskp.tile([128, NK], F32, tag="mpe")
            nc.vector.tensor_scalar_add(mpe[:], s.m_cols[:], EPS)
            s.logm2 = maskp.tile([128, NK], F32, tag="logm2")
            nc.scalar.activation(s.logm2[:], mpe[:], AF.Ln)
            nc.scalar.mul(s.logm2[:], s.logm2[:], 2.0)
            s.dn_ps = ps_dn.tile([128, 512], F32)
            # unused partitions must stay finite: 0 * inf = NaN would leak
            # through the select matmul in emit_boundary
            nc.vector.memset(s.dn_ps[:], 1.0)
            s.chunks = []

        def emit_vt(b):
            # V^T (vt[p, 128*kb + c] = V[c, 128*kb + p])
            s = st[b]
            s.vt = vtp.tile([128, L], BF16)
            for g in range(4):
                pvt = ps_vt.tile([128, 512], BF16)
                for j in range(4):
                    kb = 4 * g + j
                    nc.tensor.transpose(
                        pvt[:, j * 128 : (j + 1) * 128],
                        s.v[:, kb * 128 : (kb + 1) * 128],
                        ident[:],
                    )
                nc.scalar.copy(s.vt[:, g * 512 : (g + 1) * 512], pvt[:])

        def emit_et_exp(b, kb):
            s = st[b]
            p2t = p2tp.tile([128, L], BF16)
            s.chunks.append(p2t)
            for h in range(2):
                et = ps_et.tile([128, 1024], F32, tag="et")
                for n in range(2):
                    nc.tensor.matmul(
                        et[:, n * 512 : (n + 1) * 512],
                        lhsT=s.k[:, kb * 128 : (kb + 1) * 128],
                        rhs=s.q[:, h * 1024 + n * 512 : h * 1024 + (n + 1) * 512],
                        start=True,
                        stop=True,
                    )
                nc.scalar.activation(
                    p2t[:, h * 1024 : (h + 1) * 1024],
                    et[:],
                    AF.Exp,
                    bias=s.logm2[:, kb : kb + 1],
                    scale=SCALE,
                )

        def emit_dn(b, kb):
            s = st[b]
            for n in range(4):
                nc.tensor.matmul(
                    s.dn_ps[32 * n : 32 * n + 1, :],
                    lhsT=ones_col[:],
                    rhs=s.chunks[kb][:, n * 512 : (n + 1) * 512],
                    start=(kb == 0),
                    stop=(kb == NK - 1),
                    tile_position=(0, 32 * n),
                )

        def emit_ph1_chunk(b, kb):
            # ET/exp of chunk kb, then the denominator of chunk kb-1: the dn
            # matmuls wait on exp, so keep ready ET work ahead of them in the
            # in-order PE queue.
            emit_et_exp(b, kb)
            if kb >= 1:
                emit_dn(b, kb - 1)
            if kb == NK - 1:
                emit_dn(b, kb)

        def emit_boundary(b):
            # r = 1/denom (slices live at partitions 0/32/64/96 of dn_ps; the
            # wide reciprocal computes junk on unused partitions, which the
            # select matmul then discards), cast bf16, broadcast via PE select.
            s = st[b]
            rec32 = maskp.tile([128, 512], F32, tag="rec32")
            nc.vector.reciprocal(rec32[:], s.dn_ps[:])
            rec = maskp.tile([128, 512], BF16, tag="rec")
            nc.scalar.copy(rec[:], rec32[:])
            s.rbc = rbp.tile([128, L], BF16)
            for h in range(2):
                rb_ps = ps_et.tile([128, 1024], F32, tag="et")
                for j in range(2):
                    n = 2 * h + j
                    nc.tensor.matmul(
                        rb_ps[:, j * 512 : (j + 1) * 512],
                        lhsT=sels[n][:],
                        rhs=rec[:],
                        start=True,
                        stop=True,
                    )
                nc.scalar.copy(s.rbc[:, h * 1024 : (h + 1) * 1024], rb_ps[:])

        def emit_rmul_dma(b, kb):
            s = st[b]
            p2t = s.chunks[kb]
            nc.vector.tensor_mul(p2t[:], p2t[:], s.rbc[:])
            nc.sync.dma_start(at_d.ap()[b, kb * 128 : (kb + 1) * 128, :], p2t[:])

        def emit_av_h0(b, kb):
            s = st[b]
            if kb == 0:
                s.out_ps = ps_out.tile([128, 1024], F32, tag="out")
            for n in range(2):
                nc.tensor.matmul(
                    s.out_ps[:, n * 512 : (n + 1) * 512],
                    lhsT=s.vt[:, kb * 128 : (kb + 1) * 128],
                    rhs=s.chunks[kb][:, n * 512 : (n + 1) * 512],
                    start=(kb == 0),
                    stop=(kb == NK - 1),
                )

        AV_LAG = 3

        def emit_ph2_chunk(b, kb):
            # r-multiply of chunk kb, then the AV matmuls of chunk kb-AV_LAG:
            # the AV matmuls wait on the DVE multiply; a deep stagger keeps
            # their waits satisfied by the time they reach the PE queue head.
            emit_rmul_dma(b, kb)
            if kb >= AV_LAG:
                emit_av_h0(b, kb - AV_LAG)
            if kb == NK - 1:
                for j in range(NK - AV_LAG, NK):
                    emit_av_h0(b, j)

        def emit_av_h1_and_out(b):
            s = st[b]
            osb = outp.tile([128, L], F32)
            nc.scalar.copy(osb[:, 0:1024], s.out_ps[:])
            out_ps1 = ps_out.tile([128, 1024], F32, tag="out")
            for kb in range(NK):
                for n in range(2):
                    nc.tensor.matmul(
                        out_ps1[:, n * 512 : (n + 1) * 512],
                        lhsT=s.vt[:, kb * 128 : (kb + 1) * 128],
                        rhs=s.chunks[kb][:, 1024 + n * 512 : 1024 + (n + 1) * 512],
                        start=(kb == 0),
                        stop=(kb == NK - 1),
                    )
            nc.scalar.copy(osb[:, 1024:2048], out_ps1[:])
            nc.sync.dma_start(out_d.ap()[b], osb[:])

        # software-pipelined emission
        emit_inputs(0)
        emit_maskprep(0)
        emit_inputs(1)
        for kb in range(NK):
            emit_ph1_chunk(0, kb)
            if kb == 2:
                emit_vt(0)
        emit_boundary(0)
        emit_maskprep(1)
        for kb in range(NK):
            emit_ph2_chunk(0, kb)
            emit_ph1_chunk(1, kb)
            if kb == 2:
                emit_vt(1)
        emit_av_h1_and_out(0)
        emit_boundary(1)
        for kb in range(NK):
            emit_ph2_chunk(1, kb)
        emit_av_h1_and_out(1)

    nc.compile()
    return nc


def kernel(proj_query, proj_key, proj_val, padding_mask):
    global LAST_RESULTS
    if "nc" not in _CACHE:
        _CACHE["nc"] = _build()
    nc = _CACHE["nc"]

    proj_query = np.ascontiguousarray(np.asarray(proj_query, dtype=np.float32))
    proj_key = np.ascontiguousarray(np.asarray(proj_key, dtype=np.float32))
    proj_val = np.ascontiguousarray(np.asarray(proj_val, dtype=np.float32))
    padding_mask = np.ascontiguousarray(np.asarray(padding_mask, dtype=np.float32))

    in_maps = []
    for i in range(N_CORES):
        s = slice(i * BPC, (i + 1) * BPC)
        in_maps.append(
            {
                "q": proj_query[s],
                "k": proj_key[s],
                "v": proj_val[s],
                "mask": padding_mask[s],
            }
        )

    res = run_bass_kernel_spmd(nc, in_maps, list(range(N_CORES)))
    LAST_RESULTS = res

    out = np.concatenate([res.results[i]["out"] for i in range(N_CORES)], axis=0)
    attn_t = np.concatenate(
        [np.asarray(res.results[i]["attn_t"], dtype=np.float32) for i in range(N_CORES)],
        axis=0,
    )
    return out, attn_t
